# revision 17
# baseline (speedup 1.0000x reference)
"""GATv2 2-layer node classification on 8 Trainium2 NeuronCores (Bass/Tile).

Sharding: nodes are block-sharded across the 8 cores (6250 real + 22 pad rows
per core); every edge lives on the core that owns its dst node, sorted by dst
and packed into 49 windows of 128 dst-nodes each, padded to a fixed per-window
slot count TW so the SPMD program is identical on every core.  Per layer each
core computes xl/xr for its node shard, all-gathers xl (the gather table),
then streams its edges: indirect-DMA gathers of xl[src]/xr[dst], the GATv2
edge math on PE/DVE/ACT, and a one-hot-matmul scatter-add into per-window PSUM
accumulators.  Small weights are replicated.
"""
import numpy as np

# problem constants (hardcoded per contract)
N = 50000; E = 800000; NODE = 64; EDGE = 16; H = 2; C = 64; F = 128
L = 2; K = 10; EPS = 1e-5
P = 128
NCORE = 8
NPC = N // NCORE           # 6250
NWIN = 49
NLP = NWIN * P             # 6272 padded nodes per core
NG = NCORE * NLP           # 50176 global padded table rows

_module_cache = {}


# ----------------------------------------------------------------- host prep
def _prep_edges(edge_index, edge_attr):
    src = np.asarray(edge_index)[0].astype(np.int64)
    dst = np.asarray(edge_index)[1].astype(np.int64)
    ea = np.asarray(edge_attr).astype(np.float32)

    owner = dst // NPC
    dst_loc = dst % NPC
    src_new = (src // NPC) * NLP + (src % NPC)

    cores = []
    TW = P
    for k in range(NCORE):
        m = owner == k
        sl, sn, eak = dst_loc[m], src_new[m], ea[m]
        order = np.argsort(sl, kind="stable")
        sl, sn, eak = sl[order], sn[order], eak[order]
        cnt = np.bincount(sl // P, minlength=NWIN)
        TW = max(TW, int(np.ceil(cnt.max() / P) * P))
        cores.append((sl, sn, eak, cnt))
    EL = NWIN * TW

    per_core = []
    for k in range(NCORE):
        sl, sn, eak, cnt = cores[k]
        srcA = np.zeros(EL, np.int32)
        dstG = np.zeros(EL, np.int32)
        dstS = np.full(EL, 200.0, np.float32)
        eaT = np.zeros((EDGE, EL), np.float32)
        pos = 0
        for w in range(NWIN):
            n = int(cnt[w]); base = w * TW
            srcA[base:base + n] = sn[pos:pos + n]
            dstG[base:base + n] = sl[pos:pos + n]
            dstS[base:base + n] = (sl[pos:pos + n] - w * P).astype(np.float32)
            dstG[base + n:base + TW] = w * P
            eaT[:, base:base + n] = eak[pos:pos + n].T
            pos += n
        per_core.append(dict(src=srcA, dstG=dstG, dstS=dstS, eaT=eaT))
    return per_core, TW, EL


# ------------------------------------------------------------- device module
def _build_module(TW, EL):
    import os
    import concourse.bass as bass
    import concourse.tile as tile
    from concourse import bacc, mybir
    from concourse.masks import make_identity

    SKIP_EDGE = bool(int(os.environ.get("GAT_SKIP_EDGE", "0")))
    SKIP_IND = bool(int(os.environ.get("GAT_SKIP_IND", "0")))
    SKIP_COLL = bool(int(os.environ.get("GAT_SKIP_COLL", "0")))

    fp32 = mybir.dt.float32
    i32 = mybir.dt.int32

    nc = bacc.Bacc("TRN2", target_bir_lowering=False, debug=False,
                   num_devices=NCORE)

    # ---- I/O ----
    def di(name, shape, dt=fp32):
        return nc.dram_tensor(name, shape, dt, kind="ExternalInput")

    x_shT = di("x_shT", [NODE, NLP])            # node features, transposed
    eaT_d = di("eaT", [EDGE, EL])
    src_d = di("src", [EL], i32)
    dstG_d = di("dstG", [EL], i32)
    dstS_d = di("dstS", [EL])
    Wp_d = di("Wp", [NODE, F])
    bp_d = di("bp", [F])
    Wlg_d = di("Wlg", [L, F, F])
    blt_d = di("blt", [L, F])
    Wrg_d = di("Wrg", [L, F, F])
    brt_d = di("brt", [L, F])
    We_d = di("We", [L, EDGE, F])
    a06_d = di("a06", [L, F])
    a04_d = di("a04", [L, F])
    bout_d = di("bout", [L, F])
    Wc_d = di("Wc", [F, K])
    bc_d = di("bc", [K])

    ocls = nc.dram_tensor("ocls", [NLP, K], fp32, kind="ExternalOutput")
    oemb = nc.dram_tensor("oemb", [NLP, F], fp32, kind="ExternalOutput")

    NSUB = TW // P                      # subtiles (128 edges) per window
    GSUB = 4                            # subtiles per DVE group
    NGRP = (NSUB + GSUB - 1) // GSUB

    with tile.TileContext(nc) as tc:
        with (
            tc.tile_pool(name="singles", bufs=1) as singles,
            tc.tile_pool(name="node", bufs=3) as node_p,
            tc.tile_pool(name="nodeps", bufs=1, space="PSUM") as node_ps,
            tc.tile_pool(name="edge", bufs=3) as edge_p,
            tc.tile_pool(name="edgem", bufs=2) as edgem_p,
            tc.tile_pool(name="mps", bufs=2, space="PSUM") as m_ps,
            tc.tile_pool(name="aggps", bufs=2, space="PSUM") as agg_ps,
            tc.tile_pool(name="dram", bufs=1, space="DRAM") as dram_p,
        ):
            # ---------- static SBUF ----------
            ident = singles.tile([P, P], fp32)
            make_identity(nc, ident[:])
            iota_i = singles.tile([P, P], i32)
            nc.gpsimd.iota(iota_i[:], pattern=[[1, P]], base=0,
                           channel_multiplier=0)
            iota_f = singles.tile([P, P], fp32)
            nc.vector.tensor_copy(iota_f[:], iota_i[:])

            def bcast_row(dram_ap, width, tag):
                """[width] dram vector -> [P, width] sbuf tile (replicated)."""
                t = singles.tile([P, width], fp32, tag=tag)
                nc.sync.dma_start(
                    out=t[:],
                    in_=bass.AP(tensor=dram_ap.tensor, offset=dram_ap.offset,
                                ap=[[0, P], [1, width]]))
                return t

            Wp_sb = singles.tile([NODE, F], fp32)
            nc.sync.dma_start(out=Wp_sb[:], in_=Wp_d[:, :])
            bp_bc = bcast_row(bp_d[:], F, "bp_bc")
            Wc_sb = singles.tile([F, K], fp32)
            nc.sync.dma_start(out=Wc_sb[:], in_=Wc_d[:, :])
            bc_bc = bcast_row(bc_d[:], K, "bc_bc")

            Wlg_sb, blt_bc, Wrg_sb, brt_bc = [], [], [], []
            We_sb, a06_bc, a04_bc, bout_bc = [], [], [], []
            for l in range(L):
                t = singles.tile([F, F], fp32, tag=f"wlg{l}")
                nc.sync.dma_start(out=t[:], in_=Wlg_d[l, :, :])
                Wlg_sb.append(t)
                t = singles.tile([F, F], fp32, tag=f"wrg{l}")
                nc.sync.dma_start(out=t[:], in_=Wrg_d[l, :, :])
                Wrg_sb.append(t)
                t = singles.tile([EDGE, F], fp32, tag=f"we{l}")
                nc.sync.dma_start(out=t[:], in_=We_d[l, :, :])
                We_sb.append(t)
                blt_bc.append(bcast_row(blt_d[l, :], F, f"blt{l}"))
                brt_bc.append(bcast_row(brt_d[l, :], F, f"brt{l}"))
                bout_bc.append(bcast_row(bout_d[l, :], F, f"bout{l}"))
                # attention rows replicated across partitions, tiled x4 free
                t = singles.tile([P, GSUB, F], fp32, tag=f"a06{l}")
                nc.sync.dma_start(
                    out=t[:],
                    in_=bass.AP(tensor=a06_d[l, :].tensor,
                                offset=a06_d[l, :].offset,
                                ap=[[0, P], [0, GSUB], [1, F]]))
                a06_bc.append(t)
                t = singles.tile([P, GSUB, F], fp32, tag=f"a04{l}")
                nc.sync.dma_start(
                    out=t[:],
                    in_=bass.AP(tensor=a04_d[l, :].tensor,
                                offset=a04_d[l, :].offset,
                                ap=[[0, P], [0, GSUB], [1, F]]))
                a04_bc.append(t)

            # persistent node state: h [node%128, tile, feat]
            h_sb = singles.tile([P, NWIN, F], fp32)

            # DRAM internal tiles
            xl_in = dram_p.tile([NLP, F], fp32)
            xr_loc = dram_p.tile([NLP, F], fp32)
            xl_full = dram_p.tile([NG, F], fp32)

            # ---------- h = x @ Wp + bp ----------
            for t in range(NWIN):
                xf = node_p.tile([NODE, P], fp32, tag="xf")
                nc.sync.dma_start(out=xf[:], in_=x_shT[:, t * P:(t + 1) * P])
                ps = node_ps.tile([P, F], fp32, tag="nps")
                nc.tensor.matmul(ps[:], lhsT=xf[:], rhs=Wp_sb[:],
                                 start=True, stop=True)
                nc.vector.tensor_add(h_sb[:, t, :], ps[:], bp_bc[:])

            # ---------- layers ----------
            for l in range(L):
                # node phase: LN fold + xl/xr
                for t in range(NWIN):
                    stats = node_p.tile([P, 6], fp32, tag="bst")
                    nc.vector.bn_stats(stats[:], h_sb[:, t, :])
                    mv = node_p.tile([P, 2], fp32, tag="mv")
                    nc.vector.bn_aggr(mv[:], stats[:])
                    veps = node_p.tile([P, 1], fp32, tag="veps")
                    nc.vector.tensor_scalar_add(veps[:], mv[:, 1:2], EPS)
                    sd = node_p.tile([P, 1], fp32, tag="sd")
                    nc.scalar.activation(sd[:], veps[:],
                                         bass.mybir.ActivationFunctionType.Sqrt)
                    rstd = node_p.tile([P, 1], fp32, tag="rstd")
                    nc.vector.reciprocal(rstd[:], sd[:])
                    hstd = node_p.tile([P, F], fp32, tag="hstd")
                    nc.vector.tensor_scalar(
                        hstd[:], h_sb[:, t, :], mv[:, 0:1], rstd[:],
                        op0=bass.mybir.AluOpType.subtract,
                        op1=bass.mybir.AluOpType.mult)
                    tp = node_ps.tile([P, P], fp32, tag="tp")
                    nc.tensor.transpose(tp[:], hstd[:], ident[:])
                    hstdT = node_p.tile([P, P], fp32, tag="hstdT")
                    nc.vector.tensor_copy(hstdT[:], tp[:])
                    psl = node_ps.tile([P, F], fp32, tag="nps")
                    nc.tensor.matmul(psl[:], lhsT=hstdT[:], rhs=Wlg_sb[l][:],
                                     start=True, stop=True)
                    xl_sb = node_p.tile([P, F], fp32, tag="xlsb")
                    nc.vector.tensor_add(xl_sb[:], psl[:], blt_bc[l][:])
                    nc.sync.dma_start(out=xl_in[t * P:(t + 1) * P, :],
                                      in_=xl_sb[:])
                    psr = node_ps.tile([P, F], fp32, tag="nps")
                    nc.tensor.matmul(psr[:], lhsT=hstdT[:], rhs=Wrg_sb[l][:],
                                     start=True, stop=True)
                    xr_sb = node_p.tile([P, F], fp32, tag="xrsb")
                    nc.vector.tensor_add(xr_sb[:], psr[:], brt_bc[l][:])
                    nc.sync.dma_start(out=xr_loc[t * P:(t + 1) * P, :],
                                      in_=xr_sb[:])

                if SKIP_COLL:
                    nc.sync.dma_start(out=xl_full[0:NLP, :], in_=xl_in[:, :])
                else:
                    nc.gpsimd.collective_compute(
                        "AllGather",
                        bass.mybir.AluOpType.bypass,
                        replica_groups=[list(range(NCORE))],
                        ins=[xl_in.opt()],
                        outs=[xl_full.opt()],
                    )

                # edge phase
                for w in range(NWIN if not SKIP_EDGE else 0):
                    base = w * TW
                    sidx = edge_p.tile([P, NSUB], i32, tag="sidx")
                    nc.sync.dma_start(
                        out=sidx[:],
                        in_=bass.AP(tensor=src_d.ap().tensor, offset=base,
                                    ap=[[1, P], [P, NSUB]]))
                    gidx = edge_p.tile([P, NSUB], i32, tag="gidx")
                    nc.sync.dma_start(
                        out=gidx[:],
                        in_=bass.AP(tensor=dstG_d.ap().tensor, offset=base,
                                    ap=[[1, P], [P, NSUB]]))
                    soff = edge_p.tile([P, NSUB], fp32, tag="soff")
                    nc.sync.dma_start(
                        out=soff[:],
                        in_=bass.AP(tensor=dstS_d.ap().tensor, offset=base,
                                    ap=[[1, P], [P, NSUB]]))

                    agg = agg_ps.tile([P, F + 2], fp32, tag="agg")

                    for g in range(NGRP):
                        s0 = g * GSUB
                        gs = min(GSUB, NSUB - s0)
                        ge = gs * P
                        xls = edge_p.tile([P, GSUB, P], fp32, tag="xls")
                        xrd = edge_p.tile([P, GSUB, P], fp32, tag="xrd")
                        for s in range(gs):
                            if SKIP_IND:
                                nc.sync.dma_start(out=xls[:, s, :],
                                                  in_=xl_full[0:P, :])
                                nc.sync.dma_start(out=xrd[:, s, :],
                                                  in_=xr_loc[0:P, :])
                                continue
                            nc.gpsimd.indirect_dma_start(
                                out=xls[:, s, :], out_offset=None,
                                in_=xl_full[:, :],
                                in_offset=bass.IndirectOffsetOnAxis(
                                    ap=sidx[:, s0 + s:s0 + s + 1], axis=0))
                            nc.gpsimd.indirect_dma_start(
                                out=xrd[:, s, :], out_offset=None,
                                in_=xr_loc[:, :],
                                in_offset=bass.IndirectOffsetOnAxis(
                                    ap=gidx[:, s0 + s:s0 + s + 1], axis=0))
                        eat = edge_p.tile([EDGE, GSUB * P], fp32, tag="eat")
                        nc.sync.dma_start(
                            out=eat[:, :ge],
                            in_=eaT_d[:, base + s0 * P:base + s0 * P + ge])

                        psm = m_ps.tile([P, GSUB, P], fp32, tag="psm")
                        for s in range(gs):
                            nc.tensor.matmul(
                                psm[:, s, :],
                                lhsT=eat[:, s * P:(s + 1) * P],
                                rhs=We_sb[l][:], start=True, stop=True)

                        xv = xls[:].rearrange("p a b -> p (a b)")
                        rv = xrd[:].rearrange("p a b -> p (a b)")
                        mv_ = psm[:].rearrange("p a b -> p (a b)")
                        m_sb = edgem_p.tile([P, GSUB * P], fp32, tag="msb")
                        nc.vector.tensor_add(m_sb[:, :ge], mv_[:, :ge],
                                             xv[:, :ge])
                        nc.vector.tensor_add(m_sb[:, :ge], m_sb[:, :ge],
                                             rv[:, :ge])
                        ab = edgem_p.tile([P, GSUB * P], fp32, tag="ab")
                        nc.scalar.activation(
                            ab[:, :ge], m_sb[:, :ge],
                            bass.mybir.ActivationFunctionType.Abs)
                        t1 = edgem_p.tile([P, GSUB * P], fp32, tag="t1")
                        a06v = a06_bc[l][:].rearrange("p a b -> p (a b)")
                        a04v = a04_bc[l][:].rearrange("p a b -> p (a b)")
                        nc.vector.tensor_mul(t1[:, :ge], m_sb[:, :ge],
                                             a06v[:, :ge])
                        nc.vector.tensor_mul(ab[:, :ge], ab[:, :ge],
                                             a04v[:, :ge])
                        nc.vector.tensor_add(t1[:, :ge], t1[:, :ge],
                                             ab[:, :ge])
                        lg = edge_p.tile([P, GSUB * H], fp32, tag="lg")
                        nc.vector.reduce_sum(
                            lg[:, :gs * H],
                            t1[:, :ge].rearrange("p (s c) -> p s c", c=C),
                            axis=bass.mybir.AxisListType.X)
                        w8 = edge_p.tile([P, GSUB * H], fp32, tag="w8")
                        nc.scalar.activation(
                            w8[:, :gs * H], lg[:, :gs * H],
                            bass.mybir.ActivationFunctionType.Exp)
                        wt = edge_p.tile([P, GSUB, F + 2], fp32, tag="wt")
                        nc.vector.tensor_tensor(
                            out=wt[:, :gs, 0:F].rearrange(
                                "p s (h c) -> p s h c", h=H),
                            in0=xls[:, :gs, :].rearrange(
                                "p s (h c) -> p s h c", h=H),
                            in1=bass.AP(tensor=w8[:].tensor,
                                        offset=w8[:].offset,
                                        ap=[w8[:].ap[0], [H, gs], [1, H],
                                            [0, C]]),
                            op=bass.mybir.AluOpType.mult)
                        nc.vector.tensor_copy(
                            wt[:, :gs, F:F + 2],
                            w8[:, :gs * H].rearrange("p (s h) -> p s h", h=H))
                        S = edgem_p.tile([P, GSUB, P], fp32, tag="S")
                        nc.vector.tensor_tensor(
                            out=S[:, :gs, :],
                            in0=bass.AP(tensor=iota_f[:].tensor,
                                        offset=iota_f[:].offset,
                                        ap=[iota_f[:].ap[0], [0, gs], [1, P]]),
                            in1=bass.AP(tensor=soff[:].tensor,
                                        offset=soff[:, s0:s0 + gs].offset,
                                        ap=[soff[:].ap[0], [1, gs], [0, P]]),
                            op=bass.mybir.AluOpType.is_equal)
                        for s in range(gs):
                            nc.tensor.matmul(
                                agg[:, :], lhsT=S[:, s, :], rhs=wt[:, s, :],
                                start=(s0 + s == 0),
                                stop=(s0 + s == NSUB - 1))

                    # window close: h update
                    agg_sb = edge_p.tile([P, F + 2], fp32, tag="aggsb")
                    nc.vector.tensor_copy(agg_sb[:], agg[:])
                    nc.vector.tensor_scalar_add(agg_sb[:, F:F + 2],
                                                agg_sb[:, F:F + 2], 1e-30)
                    rec = edge_p.tile([P, H], fp32, tag="rec")
                    nc.vector.reciprocal(rec[:], agg_sb[:, F:F + 2])
                    hg = edge_p.tile([P, F], fp32, tag="hg")
                    for hh in range(H):
                        nc.vector.tensor_scalar_mul(
                            hg[:, hh * C:(hh + 1) * C],
                            agg_sb[:, hh * C:(hh + 1) * C],
                            rec[:, hh:hh + 1])
                    nc.vector.tensor_add(hg[:], hg[:], bout_bc[l][:])
                    rl = edge_p.tile([P, F], fp32, tag="rl")
                    nc.scalar.activation(
                        rl[:], hg[:], bass.mybir.ActivationFunctionType.Relu)
                    nc.vector.tensor_add(h_sb[:, w, :], rl[:], h_sb[:, w, :])

            # ---------- classifier + outputs ----------
            for t in range(NWIN):
                tp = node_ps.tile([P, P], fp32, tag="tp")
                nc.tensor.transpose(tp[:], h_sb[:, t, :], ident[:])
                hT = node_p.tile([P, P], fp32, tag="hT")
                nc.vector.tensor_copy(hT[:], tp[:])
                psc = node_ps.tile([P, K], fp32, tag="psc")
                nc.tensor.matmul(psc[:], lhsT=hT[:], rhs=Wc_sb[:],
                                 start=True, stop=True)
                cls_sb = node_p.tile([P, K], fp32, tag="clssb")
                nc.vector.tensor_add(cls_sb[:], psc[:], bc_bc[:])
                nc.sync.dma_start(out=ocls.ap()[t * P:(t + 1) * P, :],
                                  in_=cls_sb[:])
                nc.sync.dma_start(out=oemb.ap()[t * P:(t + 1) * P, :],
                                  in_=h_sb[:, t, :])

    nc.finalize()
    return nc


# ------------------------------------------------------------------- runner
def _make_in_maps(inputs, per_core):
    x = np.asarray(inputs["x"], np.float32)
    Wp = np.asarray(inputs["Wp"], np.float32)
    bp = np.asarray(inputs["bp"], np.float32)
    ln_g = np.asarray(inputs["ln_g"], np.float32)
    ln_b = np.asarray(inputs["ln_b"], np.float32)
    Wl = np.asarray(inputs["Wl"], np.float32)
    bl = np.asarray(inputs["bl"], np.float32)
    Wr = np.asarray(inputs["Wr"], np.float32)
    br = np.asarray(inputs["br"], np.float32)
    We = np.asarray(inputs["We"], np.float32)
    att = np.asarray(inputs["att"], np.float32)
    bout = np.asarray(inputs["bout"], np.float32)
    Wc = np.asarray(inputs["Wc"], np.float32)
    bc = np.asarray(inputs["bc"], np.float32)

    Wl_g = (ln_g[:, :, None] * Wl).astype(np.float32)
    bl_t = (np.einsum('lf,lfg->lg', ln_b, Wl) + bl).astype(np.float32)
    Wr_g = (ln_g[:, :, None] * Wr).astype(np.float32)
    br_t = (np.einsum('lf,lfg->lg', ln_b, Wr) + br).astype(np.float32)
    a06 = (0.6 * att.reshape(L, F)).astype(np.float32)
    a04 = (0.4 * att.reshape(L, F)).astype(np.float32)

    in_maps = []
    for k in range(NCORE):
        xp = np.zeros((NLP, NODE), np.float32)
        xp[:NPC] = x[k * NPC:(k + 1) * NPC]
        pc = per_core[k]
        in_maps.append({
            "x_shT": np.ascontiguousarray(xp.T),
            "eaT": pc["eaT"],
            "src": pc["src"],
            "dstG": pc["dstG"],
            "dstS": pc["dstS"],
            "Wp": Wp, "bp": bp,
            "Wlg": Wl_g, "blt": bl_t, "Wrg": Wr_g, "brt": br_t,
            "We": We, "a06": a06, "a04": a04, "bout": bout,
            "Wc": Wc, "bc": bc,
        })
    return in_maps


def _run(inputs, trace=False):
    from concourse import bass_utils

    per_core, TW, EL = _prep_edges(inputs["edge_index"], inputs["edge_attr"])
    key = (TW, EL)
    if key not in _module_cache:
        _module_cache[key] = _build_module(TW, EL)
    nc = _module_cache[key]
    in_maps = _make_in_maps(inputs, per_core)

    res = bass_utils.run_bass_kernel_spmd(
        nc, in_maps, core_ids=list(range(NCORE)), trace=trace)

    cls = np.concatenate([res.results[k]["ocls"][:NPC] for k in range(NCORE)])
    emb = np.concatenate([res.results[k]["oemb"][:NPC] for k in range(NCORE)])
    return (cls, emb), res


def kernel(**inputs):
    (cls, emb), _ = _run(inputs, trace=False)
    return (cls, emb)


# ---------------------------------------------------------------- benchmark
def bench(inputs, reps=6):
    """Time repeated PJRT executions of the compiled module.

    Returns (outputs, per-call wall times). Mirrors
    bass2jax.run_bass_via_pjrt's multi-core path but keeps the jitted
    executable and times each call.
    """
    import time
    import jax
    import numpy as np
    from jax.sharding import Mesh, PartitionSpec
    from jax.experimental.shard_map import shard_map
    from concourse import bass2jax, mybir
    from concourse.bass2jax import _bass_exec_p, partition_id_tensor

    bass2jax.install_neuronx_cc_hook()

    # build module + in_maps exactly as _run does
    per_core, TW, EL = _prep_edges(inputs["edge_index"], inputs["edge_attr"])
    key = (TW, EL)
    if key not in _module_cache:
        _module_cache[key] = _build_module(TW, EL)
    nc = _module_cache[key]
    in_maps = _make_in_maps(inputs, per_core)

    partition_name = (nc.partition_id_tensor.name
                      if nc.partition_id_tensor else None)
    in_names, out_names, out_avals, zero_outs = [], [], [], []
    for alloc in nc.m.functions[0].allocations:
        if not isinstance(alloc, mybir.MemoryLocationSet):
            continue
        name = alloc.memorylocations[0].name
        if alloc.kind == "ExternalInput":
            if name != partition_name:
                in_names.append(name)
        elif alloc.kind == "ExternalOutput":
            out_names.append(name)
            shape = tuple(alloc.tensor_shape)
            dtype = mybir.dt.np(alloc.dtype)
            out_avals.append(jax.core.ShapedArray(shape, dtype))
            zero_outs.append(np.zeros(shape, dtype))
    n_params = len(in_names)
    n_outs = len(out_avals)
    in_names.extend(out_names)
    if partition_name is not None:
        in_names.append(partition_name)
    donate = tuple(range(n_params, n_params + n_outs))

    def _body(*args):
        operands = list(args)
        if partition_name is not None:
            operands.append(partition_id_tensor())
        return tuple(_bass_exec_p.bind(
            *operands, out_avals=tuple(out_avals), in_names=tuple(in_names),
            out_names=tuple(out_names), lowering_input_output_aliases=(),
            sim_require_finite=True, sim_require_nnan=True, nc=nc))

    devices = jax.devices()[:NCORE]
    mesh = Mesh(np.asarray(devices), ("core",))
    sharded = jax.jit(
        shard_map(_body, mesh=mesh,
                  in_specs=(PartitionSpec("core"),) * (n_params + n_outs),
                  out_specs=(PartitionSpec("core"),) * n_outs,
                  check_rep=False),
        donate_argnums=donate, keep_unused=True)

    concat_in = [np.concatenate(
        [np.asarray(in_maps[c][in_names[i]]) for c in range(NCORE)], axis=0)
        for i in range(n_params)]
    from jax.sharding import NamedSharding
    shard = NamedSharding(mesh, PartitionSpec("core"))
    dev_in = [jax.device_put(a, shard) for a in concat_in]
    times = []
    out_arrs = None
    for _ in range(reps):
        zeros = [jax.device_put(
            np.zeros((NCORE * z.shape[0], *z.shape[1:]), z.dtype), shard)
            for z in zero_outs]
        for z in zeros:
            z.block_until_ready()
        t0 = time.perf_counter()
        out_arrs = sharded(*dev_in, *zeros)
        for o in out_arrs:
            o.block_until_ready()
        times.append(time.perf_counter() - t0)
    res = [{name: np.asarray(out_arrs[i]).reshape(NCORE, *out_avals[i].shape)[c]
            for i, name in enumerate(out_names)} for c in range(NCORE)]
    cls = np.concatenate([res[k]["ocls"][:NPC] for k in range(NCORE)])
    emb = np.concatenate([res[k]["oemb"][:NPC] for k in range(NCORE)])
    return (cls, emb), times


# revision 19
# speedup vs baseline: 29.8636x; 29.8636x over previous
"""GATv2 2-layer node classification on 8 Trainium2 NeuronCores (Bass/Tile).

Sharding: nodes are block-sharded across the 8 cores (6250 real + 22 pad rows
per core); every edge lives on the core that owns its dst node, sorted by dst
and packed into 49 windows of 128 dst-nodes each, padded to a fixed per-window
slot count TW so the SPMD program is identical on every core.  Per layer each
core computes xl/xr for its node shard, all-gathers xl (the gather table),
then streams its edges: indirect-DMA gathers of xl[src]/xr[dst], the GATv2
edge math on PE/DVE/ACT, and a one-hot-matmul scatter-add into per-window PSUM
accumulators.  Small weights are replicated.
"""
import numpy as np

# problem constants (hardcoded per contract)
N = 50000; E = 800000; NODE = 64; EDGE = 16; H = 2; C = 64; F = 128
L = 2; K = 10; EPS = 1e-5
P = 128
NCORE = 8
NPC = N // NCORE           # 6250
NWIN = 49
NLP = NWIN * P             # 6272 padded nodes per core
NG = NCORE * NLP           # 50176 global padded table rows

_module_cache = {}


# ----------------------------------------------------------------- host prep
def _prep_edges(edge_index, edge_attr):
    src = np.asarray(edge_index)[0].astype(np.int64)
    dst = np.asarray(edge_index)[1].astype(np.int64)
    ea = np.asarray(edge_attr).astype(np.float32)

    owner = dst // NPC
    dst_loc = dst % NPC
    src_new = (src // NPC) * NLP + (src % NPC)

    cores = []
    TW = P
    for k in range(NCORE):
        m = owner == k
        sl, sn, eak = dst_loc[m], src_new[m], ea[m]
        order = np.argsort(sl, kind="stable")
        sl, sn, eak = sl[order], sn[order], eak[order]
        cnt = np.bincount(sl // P, minlength=NWIN)
        TW = max(TW, int(np.ceil(cnt.max() / P) * P))
        cores.append((sl, sn, eak, cnt))
    EL = NWIN * TW

    per_core = []
    for k in range(NCORE):
        sl, sn, eak, cnt = cores[k]
        srcA = np.zeros(EL, np.int32)
        dstG = np.zeros(EL, np.int32)
        dstS = np.full(EL, 200.0, np.float32)
        eaT = np.zeros((EDGE, EL), np.float32)
        pos = 0
        for w in range(NWIN):
            n = int(cnt[w]); base = w * TW
            srcA[base:base + n] = sn[pos:pos + n]
            dstG[base:base + n] = sl[pos:pos + n]
            dstS[base:base + n] = (sl[pos:pos + n] - w * P).astype(np.float32)
            dstG[base + n:base + TW] = w * P
            eaT[:, base:base + n] = eak[pos:pos + n].T
            pos += n
        per_core.append(dict(src=srcA, dstG=dstG, dstS=dstS, eaT=eaT))
    return per_core, TW, EL


# ------------------------------------------------------------- device module
def _build_module(TW, EL):
    import os
    import concourse.bass as bass
    import concourse.tile as tile
    from concourse import bacc, mybir
    from concourse.masks import make_identity

    SKIP_EDGE = bool(int(os.environ.get("GAT_SKIP_EDGE", "0")))
    SKIP_IND = bool(int(os.environ.get("GAT_SKIP_IND", "0")))
    SKIP_COLL = bool(int(os.environ.get("GAT_SKIP_COLL", "0")))

    fp32 = mybir.dt.float32
    i32 = mybir.dt.int32

    nc = bacc.Bacc("TRN2", target_bir_lowering=False, debug=False,
                   num_devices=NCORE)

    # ---- I/O ----
    def di(name, shape, dt=fp32):
        return nc.dram_tensor(name, shape, dt, kind="ExternalInput")

    x_shT = di("x_shT", [NODE, NLP])            # node features, transposed
    eaT_d = di("eaT", [EDGE, EL])
    src_d = di("src", [EL], i32)
    dstG_d = di("dstG", [EL], i32)
    dstS_d = di("dstS", [EL])
    Wp_d = di("Wp", [NODE, F])
    bp_d = di("bp", [F])
    Wlg_d = di("Wlg", [L, F, F])
    blt_d = di("blt", [L, F])
    Wrg_d = di("Wrg", [L, F, F])
    brt_d = di("brt", [L, F])
    We_d = di("We", [L, EDGE, F])
    a06_d = di("a06", [L, F])
    a04_d = di("a04", [L, F])
    bout_d = di("bout", [L, F])
    Wc_d = di("Wc", [F, K])
    bc_d = di("bc", [K])

    ocls = nc.dram_tensor("ocls", [NLP, K], fp32, kind="ExternalOutput")
    oemb = nc.dram_tensor("oemb", [NLP, F], fp32, kind="ExternalOutput")

    NSUB = TW // P                      # subtiles (128 edges) per window
    GSUB = 4                            # subtiles per DVE group
    NGRP = (NSUB + GSUB - 1) // GSUB

    with tile.TileContext(nc) as tc:
        with (
            tc.tile_pool(name="singles", bufs=1) as singles,
            tc.tile_pool(name="node", bufs=3) as node_p,
            tc.tile_pool(name="nodeps", bufs=1, space="PSUM") as node_ps,
            tc.tile_pool(name="edge", bufs=3) as edge_p,
            tc.tile_pool(name="edgem", bufs=2) as edgem_p,
            tc.tile_pool(name="mps", bufs=2, space="PSUM") as m_ps,
            tc.tile_pool(name="aggps", bufs=2, space="PSUM") as agg_ps,
            tc.tile_pool(name="dram", bufs=1, space="DRAM") as dram_p,
        ):
            # ---------- static SBUF ----------
            ident = singles.tile([P, P], fp32)
            make_identity(nc, ident[:])
            iota_i = singles.tile([P, P], i32)
            nc.gpsimd.iota(iota_i[:], pattern=[[1, P]], base=0,
                           channel_multiplier=0)
            iota_f = singles.tile([P, P], fp32)
            nc.vector.tensor_copy(iota_f[:], iota_i[:])

            def bcast_row(dram_ap, width, tag):
                """[width] dram vector -> [P, width] sbuf tile (replicated)."""
                t = singles.tile([P, width], fp32, tag=tag)
                nc.sync.dma_start(
                    out=t[:],
                    in_=bass.AP(tensor=dram_ap.tensor, offset=dram_ap.offset,
                                ap=[[0, P], [1, width]]))
                return t

            Wp_sb = singles.tile([NODE, F], fp32)
            nc.sync.dma_start(out=Wp_sb[:], in_=Wp_d[:, :])
            bp_bc = bcast_row(bp_d[:], F, "bp_bc")
            Wc_sb = singles.tile([F, K], fp32)
            nc.sync.dma_start(out=Wc_sb[:], in_=Wc_d[:, :])
            bc_bc = bcast_row(bc_d[:], K, "bc_bc")

            Wlg_sb, blt_bc, Wrg_sb, brt_bc = [], [], [], []
            We_sb, a06_bc, a04_bc, bout_bc = [], [], [], []
            for l in range(L):
                t = singles.tile([F, F], fp32, tag=f"wlg{l}")
                nc.sync.dma_start(out=t[:], in_=Wlg_d[l, :, :])
                Wlg_sb.append(t)
                t = singles.tile([F, F], fp32, tag=f"wrg{l}")
                nc.sync.dma_start(out=t[:], in_=Wrg_d[l, :, :])
                Wrg_sb.append(t)
                t = singles.tile([EDGE, F], fp32, tag=f"we{l}")
                nc.sync.dma_start(out=t[:], in_=We_d[l, :, :])
                We_sb.append(t)
                blt_bc.append(bcast_row(blt_d[l, :], F, f"blt{l}"))
                brt_bc.append(bcast_row(brt_d[l, :], F, f"brt{l}"))
                bout_bc.append(bcast_row(bout_d[l, :], F, f"bout{l}"))
                # attention rows replicated across partitions, tiled x4 free
                t = singles.tile([P, GSUB, F], fp32, tag=f"a06{l}")
                nc.sync.dma_start(
                    out=t[:],
                    in_=bass.AP(tensor=a06_d[l, :].tensor,
                                offset=a06_d[l, :].offset,
                                ap=[[0, P], [0, GSUB], [1, F]]))
                a06_bc.append(t)
                t = singles.tile([P, GSUB, F], fp32, tag=f"a04{l}")
                nc.sync.dma_start(
                    out=t[:],
                    in_=bass.AP(tensor=a04_d[l, :].tensor,
                                offset=a04_d[l, :].offset,
                                ap=[[0, P], [0, GSUB], [1, F]]))
                a04_bc.append(t)

            # persistent node state: h [node%128, tile, feat]
            h_sb = singles.tile([P, NWIN, F], fp32)

            # DRAM internal tiles
            xl_in = dram_p.tile([NLP, F], fp32)
            xr_loc = dram_p.tile([NLP, F], fp32)
            xl_full = dram_p.tile([NG, F], fp32)

            # ---------- h = x @ Wp + bp ----------
            for t in range(NWIN):
                xf = node_p.tile([NODE, P], fp32, tag="xf")
                nc.sync.dma_start(out=xf[:], in_=x_shT[:, t * P:(t + 1) * P])
                ps = node_ps.tile([P, F], fp32, tag="nps")
                nc.tensor.matmul(ps[:], lhsT=xf[:], rhs=Wp_sb[:],
                                 start=True, stop=True)
                nc.vector.tensor_add(h_sb[:, t, :], ps[:], bp_bc[:])

            # ---------- layers ----------
            for l in range(L):
                # node phase: LN fold + xl/xr
                for t in range(NWIN):
                    stats = node_p.tile([P, 6], fp32, tag="bst")
                    nc.vector.bn_stats(stats[:], h_sb[:, t, :])
                    mv = node_p.tile([P, 2], fp32, tag="mv")
                    nc.vector.bn_aggr(mv[:], stats[:])
                    veps = node_p.tile([P, 1], fp32, tag="veps")
                    nc.vector.tensor_scalar_add(veps[:], mv[:, 1:2], EPS)
                    sd = node_p.tile([P, 1], fp32, tag="sd")
                    nc.scalar.activation(sd[:], veps[:],
                                         bass.mybir.ActivationFunctionType.Sqrt)
                    rstd = node_p.tile([P, 1], fp32, tag="rstd")
                    nc.vector.reciprocal(rstd[:], sd[:])
                    hstd = node_p.tile([P, F], fp32, tag="hstd")
                    nc.vector.tensor_scalar(
                        hstd[:], h_sb[:, t, :], mv[:, 0:1], rstd[:],
                        op0=bass.mybir.AluOpType.subtract,
                        op1=bass.mybir.AluOpType.mult)
                    tp = node_ps.tile([P, P], fp32, tag="tp")
                    nc.tensor.transpose(tp[:], hstd[:], ident[:])
                    hstdT = node_p.tile([P, P], fp32, tag="hstdT")
                    nc.vector.tensor_copy(hstdT[:], tp[:])
                    psl = node_ps.tile([P, F], fp32, tag="nps")
                    nc.tensor.matmul(psl[:], lhsT=hstdT[:], rhs=Wlg_sb[l][:],
                                     start=True, stop=True)
                    xl_sb = node_p.tile([P, F], fp32, tag="xlsb")
                    nc.vector.tensor_add(xl_sb[:], psl[:], blt_bc[l][:])
                    nc.sync.dma_start(out=xl_in[t * P:(t + 1) * P, :],
                                      in_=xl_sb[:])
                    psr = node_ps.tile([P, F], fp32, tag="nps")
                    nc.tensor.matmul(psr[:], lhsT=hstdT[:], rhs=Wrg_sb[l][:],
                                     start=True, stop=True)
                    xr_sb = node_p.tile([P, F], fp32, tag="xrsb")
                    nc.vector.tensor_add(xr_sb[:], psr[:], brt_bc[l][:])
                    nc.sync.dma_start(out=xr_loc[t * P:(t + 1) * P, :],
                                      in_=xr_sb[:])

                if SKIP_COLL:
                    nc.sync.dma_start(out=xl_full[0:NLP, :], in_=xl_in[:, :])
                else:
                    nc.gpsimd.collective_compute(
                        "AllGather",
                        bass.mybir.AluOpType.bypass,
                        replica_groups=[list(range(NCORE))],
                        ins=[xl_in.opt()],
                        outs=[xl_full.opt()],
                    )

                # edge phase
                for w in range(NWIN if not SKIP_EDGE else 0):
                    base = w * TW
                    sidx = edge_p.tile([P, NSUB], i32, tag="sidx")
                    nc.sync.dma_start(
                        out=sidx[:],
                        in_=bass.AP(tensor=src_d.ap().tensor, offset=base,
                                    ap=[[1, P], [P, NSUB]]))
                    gidx = edge_p.tile([P, NSUB], i32, tag="gidx")
                    nc.sync.dma_start(
                        out=gidx[:],
                        in_=bass.AP(tensor=dstG_d.ap().tensor, offset=base,
                                    ap=[[1, P], [P, NSUB]]))
                    soff = edge_p.tile([P, NSUB], fp32, tag="soff")
                    nc.sync.dma_start(
                        out=soff[:],
                        in_=bass.AP(tensor=dstS_d.ap().tensor, offset=base,
                                    ap=[[1, P], [P, NSUB]]))

                    agg = agg_ps.tile([P, F + 2], fp32, tag="agg")

                    for g in range(NGRP):
                        s0 = g * GSUB
                        gs = min(GSUB, NSUB - s0)
                        ge = gs * P
                        xls = edge_p.tile([P, GSUB, P], fp32, tag="xls")
                        xrd = edge_p.tile([P, GSUB, P], fp32, tag="xrd")
                        for s in range(gs):
                            if SKIP_IND:
                                nc.sync.dma_start(out=xls[:, s, :],
                                                  in_=xl_full[0:P, :])
                                nc.sync.dma_start(out=xrd[:, s, :],
                                                  in_=xr_loc[0:P, :])
                                continue
                            nc.gpsimd.indirect_dma_start(
                                out=xls[:, s, :], out_offset=None,
                                in_=xl_full[:, :],
                                in_offset=bass.IndirectOffsetOnAxis(
                                    ap=sidx[:, s0 + s:s0 + s + 1], axis=0))
                            nc.gpsimd.indirect_dma_start(
                                out=xrd[:, s, :], out_offset=None,
                                in_=xr_loc[:, :],
                                in_offset=bass.IndirectOffsetOnAxis(
                                    ap=gidx[:, s0 + s:s0 + s + 1], axis=0))
                        eat = edge_p.tile([EDGE, GSUB * P], fp32, tag="eat")
                        nc.sync.dma_start(
                            out=eat[:, :ge],
                            in_=eaT_d[:, base + s0 * P:base + s0 * P + ge])

                        psm = m_ps.tile([P, GSUB, P], fp32, tag="psm")
                        for s in range(gs):
                            nc.tensor.matmul(
                                psm[:, s, :],
                                lhsT=eat[:, s * P:(s + 1) * P],
                                rhs=We_sb[l][:], start=True, stop=True)

                        xv = xls[:].rearrange("p a b -> p (a b)")
                        rv = xrd[:].rearrange("p a b -> p (a b)")
                        mv_ = psm[:].rearrange("p a b -> p (a b)")
                        m_sb = edgem_p.tile([P, GSUB * P], fp32, tag="msb")
                        nc.vector.tensor_add(m_sb[:, :ge], mv_[:, :ge],
                                             xv[:, :ge])
                        nc.vector.tensor_add(m_sb[:, :ge], m_sb[:, :ge],
                                             rv[:, :ge])
                        ab = edgem_p.tile([P, GSUB * P], fp32, tag="ab")
                        nc.scalar.activation(
                            ab[:, :ge], m_sb[:, :ge],
                            bass.mybir.ActivationFunctionType.Abs)
                        t1 = edgem_p.tile([P, GSUB * P], fp32, tag="t1")
                        a06v = a06_bc[l][:].rearrange("p a b -> p (a b)")
                        a04v = a04_bc[l][:].rearrange("p a b -> p (a b)")
                        nc.vector.tensor_mul(t1[:, :ge], m_sb[:, :ge],
                                             a06v[:, :ge])
                        nc.vector.tensor_mul(ab[:, :ge], ab[:, :ge],
                                             a04v[:, :ge])
                        nc.vector.tensor_add(t1[:, :ge], t1[:, :ge],
                                             ab[:, :ge])
                        lg = edge_p.tile([P, GSUB * H], fp32, tag="lg")
                        nc.vector.reduce_sum(
                            lg[:, :gs * H],
                            t1[:, :ge].rearrange("p (s c) -> p s c", c=C),
                            axis=bass.mybir.AxisListType.X)
                        w8 = edge_p.tile([P, GSUB * H], fp32, tag="w8")
                        nc.scalar.activation(
                            w8[:, :gs * H], lg[:, :gs * H],
                            bass.mybir.ActivationFunctionType.Exp)
                        wt = edge_p.tile([P, GSUB, F + 2], fp32, tag="wt")
                        nc.vector.tensor_tensor(
                            out=wt[:, :gs, 0:F].rearrange(
                                "p s (h c) -> p s h c", h=H),
                            in0=xls[:, :gs, :].rearrange(
                                "p s (h c) -> p s h c", h=H),
                            in1=bass.AP(tensor=w8[:].tensor,
                                        offset=w8[:].offset,
                                        ap=[w8[:].ap[0], [H, gs], [1, H],
                                            [0, C]]),
                            op=bass.mybir.AluOpType.mult)
                        nc.vector.tensor_copy(
                            wt[:, :gs, F:F + 2],
                            w8[:, :gs * H].rearrange("p (s h) -> p s h", h=H))
                        S = edgem_p.tile([P, GSUB, P], fp32, tag="S")
                        nc.vector.tensor_tensor(
                            out=S[:, :gs, :],
                            in0=bass.AP(tensor=iota_f[:].tensor,
                                        offset=iota_f[:].offset,
                                        ap=[iota_f[:].ap[0], [0, gs], [1, P]]),
                            in1=bass.AP(tensor=soff[:].tensor,
                                        offset=soff[:, s0:s0 + gs].offset,
                                        ap=[soff[:].ap[0], [1, gs], [0, P]]),
                            op=bass.mybir.AluOpType.is_equal)
                        for s in range(gs):
                            nc.tensor.matmul(
                                agg[:, :], lhsT=S[:, s, :], rhs=wt[:, s, :],
                                start=(s0 + s == 0),
                                stop=(s0 + s == NSUB - 1))

                    # window close: h update
                    agg_sb = edge_p.tile([P, F + 2], fp32, tag="aggsb")
                    nc.vector.tensor_copy(agg_sb[:], agg[:])
                    nc.vector.tensor_scalar_add(agg_sb[:, F:F + 2],
                                                agg_sb[:, F:F + 2], 1e-30)
                    rec = edge_p.tile([P, H], fp32, tag="rec")
                    nc.vector.reciprocal(rec[:], agg_sb[:, F:F + 2])
                    hg = edge_p.tile([P, F], fp32, tag="hg")
                    for hh in range(H):
                        nc.vector.tensor_scalar_mul(
                            hg[:, hh * C:(hh + 1) * C],
                            agg_sb[:, hh * C:(hh + 1) * C],
                            rec[:, hh:hh + 1])
                    nc.vector.tensor_add(hg[:], hg[:], bout_bc[l][:])
                    rl = edge_p.tile([P, F], fp32, tag="rl")
                    nc.scalar.activation(
                        rl[:], hg[:], bass.mybir.ActivationFunctionType.Relu)
                    nc.vector.tensor_add(h_sb[:, w, :], rl[:], h_sb[:, w, :])

            # ---------- classifier + outputs ----------
            for t in range(NWIN):
                tp = node_ps.tile([P, P], fp32, tag="tp")
                nc.tensor.transpose(tp[:], h_sb[:, t, :], ident[:])
                hT = node_p.tile([P, P], fp32, tag="hT")
                nc.vector.tensor_copy(hT[:], tp[:])
                psc = node_ps.tile([P, K], fp32, tag="psc")
                nc.tensor.matmul(psc[:], lhsT=hT[:], rhs=Wc_sb[:],
                                 start=True, stop=True)
                cls_sb = node_p.tile([P, K], fp32, tag="clssb")
                nc.vector.tensor_add(cls_sb[:], psc[:], bc_bc[:])
                nc.sync.dma_start(out=ocls.ap()[t * P:(t + 1) * P, :],
                                  in_=cls_sb[:])
                nc.sync.dma_start(out=oemb.ap()[t * P:(t + 1) * P, :],
                                  in_=h_sb[:, t, :])

    nc.finalize()
    return nc


# ------------------------------------------------------------------- runner
def _make_in_maps(inputs, per_core):
    x = np.asarray(inputs["x"], np.float32)
    Wp = np.asarray(inputs["Wp"], np.float32)
    bp = np.asarray(inputs["bp"], np.float32)
    ln_g = np.asarray(inputs["ln_g"], np.float32)
    ln_b = np.asarray(inputs["ln_b"], np.float32)
    Wl = np.asarray(inputs["Wl"], np.float32)
    bl = np.asarray(inputs["bl"], np.float32)
    Wr = np.asarray(inputs["Wr"], np.float32)
    br = np.asarray(inputs["br"], np.float32)
    We = np.asarray(inputs["We"], np.float32)
    att = np.asarray(inputs["att"], np.float32)
    bout = np.asarray(inputs["bout"], np.float32)
    Wc = np.asarray(inputs["Wc"], np.float32)
    bc = np.asarray(inputs["bc"], np.float32)

    Wl_g = (ln_g[:, :, None] * Wl).astype(np.float32)
    bl_t = (np.einsum('lf,lfg->lg', ln_b, Wl) + bl).astype(np.float32)
    Wr_g = (ln_g[:, :, None] * Wr).astype(np.float32)
    br_t = (np.einsum('lf,lfg->lg', ln_b, Wr) + br).astype(np.float32)
    a06 = (0.6 * att.reshape(L, F)).astype(np.float32)
    a04 = (0.4 * att.reshape(L, F)).astype(np.float32)

    in_maps = []
    for k in range(NCORE):
        xp = np.zeros((NLP, NODE), np.float32)
        xp[:NPC] = x[k * NPC:(k + 1) * NPC]
        pc = per_core[k]
        in_maps.append({
            "x_shT": np.ascontiguousarray(xp.T),
            "eaT": pc["eaT"],
            "src": pc["src"],
            "dstG": pc["dstG"],
            "dstS": pc["dstS"],
            "Wp": Wp, "bp": bp,
            "Wlg": Wl_g, "blt": bl_t, "Wrg": Wr_g, "brt": br_t,
            "We": We, "a06": a06, "a04": a04, "bout": bout,
            "Wc": Wc, "bc": bc,
        })
    return in_maps


def _run(inputs, trace=False):
    from concourse import bass_utils

    per_core, TW, EL = _prep_edges(inputs["edge_index"], inputs["edge_attr"])
    import os as _os
    key = (TW, EL, _os.environ.get("GAT_SKIP_EDGE"),
           _os.environ.get("GAT_SKIP_IND"), _os.environ.get("GAT_SKIP_COLL"))
    if key not in _module_cache:
        _module_cache[key] = _build_module(TW, EL)
    nc = _module_cache[key]
    in_maps = _make_in_maps(inputs, per_core)

    res = bass_utils.run_bass_kernel_spmd(
        nc, in_maps, core_ids=list(range(NCORE)), trace=trace)

    cls = np.concatenate([res.results[k]["ocls"][:NPC] for k in range(NCORE)])
    emb = np.concatenate([res.results[k]["oemb"][:NPC] for k in range(NCORE)])
    return (cls, emb), res


def kernel(**inputs):
    (cls, emb), _ = _run(inputs, trace=False)
    return (cls, emb)


# ---------------------------------------------------------------- benchmark
def bench(inputs, reps=6):
    """Time repeated PJRT executions of the compiled module.

    Returns (outputs, per-call wall times). Mirrors
    bass2jax.run_bass_via_pjrt's multi-core path but keeps the jitted
    executable and times each call.
    """
    import time
    import jax
    import numpy as np
    from jax.sharding import Mesh, PartitionSpec
    from jax.experimental.shard_map import shard_map
    from concourse import bass2jax, mybir
    from concourse.bass2jax import _bass_exec_p, partition_id_tensor

    bass2jax.install_neuronx_cc_hook()

    # build module + in_maps exactly as _run does
    import os as _os
    per_core, TW, EL = _prep_edges(inputs["edge_index"], inputs["edge_attr"])
    key = (TW, EL, _os.environ.get("GAT_SKIP_EDGE"),
           _os.environ.get("GAT_SKIP_IND"), _os.environ.get("GAT_SKIP_COLL"))
    if key not in _module_cache:
        _module_cache[key] = _build_module(TW, EL)
    nc = _module_cache[key]
    in_maps = _make_in_maps(inputs, per_core)

    partition_name = (nc.partition_id_tensor.name
                      if nc.partition_id_tensor else None)
    in_names, out_names, out_avals, zero_outs = [], [], [], []
    for alloc in nc.m.functions[0].allocations:
        if not isinstance(alloc, mybir.MemoryLocationSet):
            continue
        name = alloc.memorylocations[0].name
        if alloc.kind == "ExternalInput":
            if name != partition_name:
                in_names.append(name)
        elif alloc.kind == "ExternalOutput":
            out_names.append(name)
            shape = tuple(alloc.tensor_shape)
            dtype = mybir.dt.np(alloc.dtype)
            out_avals.append(jax.core.ShapedArray(shape, dtype))
            zero_outs.append(np.zeros(shape, dtype))
    n_params = len(in_names)
    n_outs = len(out_avals)
    in_names.extend(out_names)
    if partition_name is not None:
        in_names.append(partition_name)
    donate = tuple(range(n_params, n_params + n_outs))

    def _body(*args):
        operands = list(args)
        if partition_name is not None:
            operands.append(partition_id_tensor())
        return tuple(_bass_exec_p.bind(
            *operands, out_avals=tuple(out_avals), in_names=tuple(in_names),
            out_names=tuple(out_names), lowering_input_output_aliases=(),
            sim_require_finite=True, sim_require_nnan=True, nc=nc))

    devices = jax.devices()[:NCORE]
    mesh = Mesh(np.asarray(devices), ("core",))
    sharded = jax.jit(
        shard_map(_body, mesh=mesh,
                  in_specs=(PartitionSpec("core"),) * (n_params + n_outs),
                  out_specs=(PartitionSpec("core"),) * n_outs,
                  check_rep=False),
        donate_argnums=donate, keep_unused=True)

    concat_in = [np.concatenate(
        [np.asarray(in_maps[c][in_names[i]]) for c in range(NCORE)], axis=0)
        for i in range(n_params)]
    from jax.sharding import NamedSharding
    shard = NamedSharding(mesh, PartitionSpec("core"))
    dev_in = [jax.device_put(a, shard) for a in concat_in]
    times = []
    out_arrs = None
    for _ in range(reps):
        zeros = [jax.device_put(
            np.zeros((NCORE * z.shape[0], *z.shape[1:]), z.dtype), shard)
            for z in zero_outs]
        for z in zeros:
            z.block_until_ready()
        t0 = time.perf_counter()
        out_arrs = sharded(*dev_in, *zeros)
        for o in out_arrs:
            o.block_until_ready()
        times.append(time.perf_counter() - t0)
    res = [{name: np.asarray(out_arrs[i]).reshape(NCORE, *out_avals[i].shape)[c]
            for i, name in enumerate(out_names)} for c in range(NCORE)]
    cls = np.concatenate([res[k]["ocls"][:NPC] for k in range(NCORE)])
    emb = np.concatenate([res[k]["oemb"][:NPC] for k in range(NCORE)])
    return (cls, emb), times
